# revision 25
# baseline (speedup 1.0000x reference)
"""Trainium2 Bass kernel for nn_AttentionLayer (4x2048x768, d_k=128, d_v=768).

Sharding (sequence-parallel over keys, data-parallel over batch):
8 cores; core c handles batch b=c//2 with KEY half h=c%2. Each core computes
q for ALL 2048 queries but k/v only for its own 1024 keys, then produces the
partial (unnormalized) attention numerator plus the partial softmax row sum:

    out_core[q, 0:768] = sum_{t in own half} exp(s_qt) * v[t, :]
    out_core[q, 768]   = sum_{t in own half} exp(s_qt)

The host adds the two partials of each batch and normalizes
(out = num/rowsum + bv) — an exact reassociation of the softmax.

x[b] is passed TRANSPOSED and t-rotated so the core's own key half is always
columns 0:1024 (one SPMD program serves all cores); query rows come back in
the rotated order and are un-rotated on the host.

Matmul dtype per stage: float32r (TF32-like, 1 PE cycle/row at N>=256;
rounding happens inside the PE on operand read — SBUF bytes stay fp32).
Set ATTN_MM_MODE=f32 for exact-fp32 matmuls (4 cycles/row).
"""

import sys

sys.path.insert(0, "/opt/trn_rl_repo")

import numpy as np

B, T, DIN, DK, DV = 4, 2048, 768, 128, 768
NCORES = 8
TOWN = 1024  # own keys per core
CH = DIN // 128  # 6 contraction chunks over d_in
TCH = TOWN // 128  # 8 own-key chunks
QCH = T // 128  # 16 query chunks (all queries)
SCALE = 1.0 / float(np.sqrt(DK))

import os as _os

_MODE = _os.environ.get("ATTN_MM_MODE", "f32r")  # "f32" | "f32r"
_R = _MODE == "f32r"

_CACHE = {}


def _build():
    from contextlib import ExitStack

    from concourse import bacc, mybir, tile

    f32 = mybir.dt.float32
    f32r = mybir.dt.float32r

    def rr(ap, on=True):
        return ap.bitcast(f32r) if (on and _R) else ap

    nc = bacc.Bacc("TRN2", target_bir_lowering=False, debug=False)

    xT = nc.dram_tensor("xT", [DIN, T], f32, kind="ExternalInput").ap()
    wq = nc.dram_tensor("wq", [DIN, DK], f32, kind="ExternalInput").ap()
    wk = nc.dram_tensor("wk", [DIN, DK], f32, kind="ExternalInput").ap()
    wv = nc.dram_tensor("wv", [DIN, DV], f32, kind="ExternalInput").ap()
    bq = nc.dram_tensor("bq", [DK, 1], f32, kind="ExternalInput").ap()
    bk = nc.dram_tensor("bk", [DK, 1], f32, kind="ExternalInput").ap()
    out = nc.dram_tensor("out", [T, DV + 1], f32, kind="ExternalOutput").ap()

    with tile.TileContext(nc) as tc, ExitStack() as ctx:
        consts = ctx.enter_context(tc.tile_pool(name="consts", bufs=1))
        persist = ctx.enter_context(tc.tile_pool(name="persist", bufs=1))
        wpool = ctx.enter_context(tc.tile_pool(name="wpool", bufs=1))
        xpool = ctx.enter_context(tc.tile_pool(name="xpool", bufs=1))
        out_pool = ctx.enter_context(tc.tile_pool(name="out_pool", bufs=4))
        ps_pool = ctx.enter_context(tc.tile_pool(name="ps", bufs=4, space="PSUM"))

        bq_sb = consts.tile([DK, 1], f32)
        bk_sb = consts.tile([DK, 1], f32)
        nc.gpsimd.dma_start(out=bq_sb[:], in_=bq)
        nc.gpsimd.dma_start(out=bk_sb[:], in_=bk)

        qT_sb = persist.tile([128, T], f32)  # [dk, q] all queries
        kT_sb = persist.tile([128, TOWN], f32)  # [dk, t-own]
        v_sb = persist.tile([128, TCH, DV + 2], f32)  # [t-part, chunk, dv|1|pad]
        pT_sb = persist.tile([128, TCH, T], f32)  # [t-part, chunk, q]

        nc.vector.memset(v_sb[:, :, DV : DV + 2], 1.0)

        xT_sb = xpool.tile([128, CH, T], f32)
        wq_sb = wpool.tile([128, CH, DK], f32)
        wk_sb = wpool.tile([128, CH, DK], f32)
        wv_sb = wpool.tile([128, CH, DV], f32)
        xT_r = xT.rearrange("(c p) t -> p c t", p=128)
        # DMA order: small weights on the scalar HWDGE queue; sync ring
        # carries (own-x chunks, wv, other-x chunks) in FIFO order so bytes
        # land in the order the PE consumes them.
        wq_r = wq.rearrange("(c p) k -> p c k", p=128)
        wk_r = wk.rearrange("(c p) k -> p c k", p=128)
        # per-chunk weight loads: MM #1 needs only wq[c0] (64KB), not all of wq
        for c in range(CH):
            nc.scalar.dma_start(out=rr(wq_sb[:, c]), in_=rr(wq_r[:, c]))
            nc.scalar.dma_start(out=rr(wk_sb[:, c]), in_=rr(wk_r[:, c]))
        # x own-half: fine-grained first chunk for an early PE start, whole
        # chunks after (HWDGE issue slices serialize ~0.75us each on sync)
        for n0 in range(0, TOWN, 512):
            nc.sync.dma_start(
                out=rr(xT_sb[:, 0, n0 : n0 + 512]),
                in_=rr(xT_r[:, 0, n0 : n0 + 512]),
            )
        for c in range(1, CH):
            nc.sync.dma_start(
                out=rr(xT_sb[:, c, 0:TOWN]), in_=rr(xT_r[:, c, 0:TOWN])
            )
        nc.sync.dma_start(
            out=rr(wv_sb[:]), in_=rr(wv.rearrange("(c p) k -> p c k", p=128))
        )
        for c in range(CH):
            nc.sync.dma_start(out=rr(xT_sb[:, c, TOWN:T]), in_=rr(xT_r[:, c, TOWN:T]))

        def emit_scores(qh):
            # scores^T per own-key chunk then P^T = exp(scale*s)
            for t in range(TCH):
                ps_s = ps_pool.tile([128, 1024], f32, tag="ps")
                for n0 in range(0, 1024, 512):
                    nc.tensor.matmul(
                        ps_s[:, n0 : n0 + 512],
                        rr(kT_sb[:, t * 128 : (t + 1) * 128]),
                        rr(qT_sb[:, qh * 1024 + n0 : qh * 1024 + n0 + 512]),
                        start=True,
                        stop=True,
                    )
                nc.scalar.activation(
                    rr(pT_sb[:, t, qh * 1024 : (qh + 1) * 1024]),
                    ps_s[:],
                    mybir.ActivationFunctionType.Exp,
                    scale=SCALE,
                )

        def emit_out(qh):
            # partial numerator + rowsum: out[qc] = sum_t P^T[t,qc].T @ [v|1];
            # copy+store each 512-col region as soon as its accumulation stops
            # so the final DMA overlaps the next region's matmuls
            for qc in range(qh * QCH // 2, (qh + 1) * QCH // 2):
                ps_o = ps_pool.tile([128, 1024], f32, tag="ps")
                o_sb = out_pool.tile([128, DV + 1], f32, tag="o")
                for n0, n1 in ((0, 512), (512, DV + 2)):
                    for t in range(TCH):
                        nc.tensor.matmul(
                            ps_o[:, n0:n1],
                            rr(pT_sb[:, t, qc * 128 : (qc + 1) * 128]),
                            rr(v_sb[:, t, n0:n1]),
                            start=(t == 0),
                            stop=(t == TCH - 1),
                        )
                    c1 = min(n1, DV + 1)
                    nc.vector.tensor_copy(o_sb[:, n0:c1], ps_o[:, n0:c1])
                    nc.sync.dma_start(
                        out=out[qc * 128 : (qc + 1) * 128, n0:c1],
                        in_=o_sb[:, n0:c1],
                    )

        # q own-half + k own, c-outer (PE consumes chunks as they stream)
        ps_q0 = ps_pool.tile([128, 1024], f32, tag="ps")
        ps_k = ps_pool.tile([128, 1024], f32, tag="ps")
        for c in range(CH):
            for n0 in range(0, TOWN, 512):
                nc.tensor.matmul(
                    ps_q0[:, n0 : n0 + 512],
                    rr(wq_sb[:, c, :]),
                    rr(xT_sb[:, c, n0 : n0 + 512]),
                    start=(c == 0),
                    stop=(c == CH - 1),
                )
                nc.tensor.matmul(
                    ps_k[:, n0 : n0 + 512],
                    rr(wk_sb[:, c, :]),
                    rr(xT_sb[:, c, n0 : n0 + 512]),
                    start=(c == 0),
                    stop=(c == CH - 1),
                )
        # split the bias-copies so scores t=0 unblocks as early as possible:
        # it needs only kT[:,0:128] and qT[:,0:512]
        for lo, hi in ((0, 128), (128, TOWN)):
            nc.scalar.activation(
                rr(kT_sb[:, lo:hi]),
                ps_k[:, lo:hi],
                mybir.ActivationFunctionType.Identity,
                bias=bk_sb[:],
            )
            nc.scalar.activation(
                rr(qT_sb[:, lo * 4 : min(hi * 4, TOWN)]),
                ps_q0[:, lo * 4 : min(hi * 4, TOWN)],
                mybir.ActivationFunctionType.Identity,
                bias=bq_sb[:],
            )

        # own-query scores need only q/k-own — run while wv/other-x stream
        emit_scores(0)

        # v for own keys
        for t in range(TCH):
            ps_v = ps_pool.tile([128, 1024], f32, tag="ps")
            for c in range(CH):
                for n0, n1 in ((0, 512), (512, DV)):
                    nc.tensor.matmul(
                        ps_v[:, n0:n1],
                        rr(xT_sb[:, c, t * 128 : (t + 1) * 128]),
                        rr(wv_sb[:, c, n0:n1]),
                        start=(c == 0),
                        stop=(c == CH - 1),
                    )
            nc.vector.tensor_copy(rr(v_sb[:, t, 0:DV]), ps_v[:, 0:DV])

        # q other-half (x tail has landed by now; its ACT overlaps out-qh0)
        ps_q1 = ps_pool.tile([128, 1024], f32, tag="ps")
        for c in range(CH):
            for n0 in range(0, TOWN, 512):
                nc.tensor.matmul(
                    ps_q1[:, n0 : n0 + 512],
                    rr(wq_sb[:, c, :]),
                    rr(xT_sb[:, c, TOWN + n0 : TOWN + n0 + 512]),
                    start=(c == 0),
                    stop=(c == CH - 1),
                )
        nc.scalar.activation(
            rr(qT_sb[:, TOWN:T]),
            ps_q1[:],
            mybir.ActivationFunctionType.Identity,
            bias=bq_sb[:],
        )

        # first output half while remaining bytes stream
        emit_out(0)

        emit_scores(1)
        emit_out(1)

    nc.compile()
    return nc


def _get_nc():
    if "nc" not in _CACHE:
        _CACHE["nc"] = _build()
    return _CACHE["nc"]


def _make_in_maps(x, Wq, bq, Wk, bk, Wv):
    base = {
        "wq": np.ascontiguousarray(Wq, dtype=np.float32),
        "wk": np.ascontiguousarray(Wk, dtype=np.float32),
        "wv": np.ascontiguousarray(Wv, dtype=np.float32),
        "bq": np.ascontiguousarray(np.asarray(bq, np.float32).reshape(DK, 1)),
        "bk": np.ascontiguousarray(np.asarray(bk, np.float32).reshape(DK, 1)),
    }
    in_maps = []
    for c in range(NCORES):
        b, h = c // 2, c % 2
        xb = x[b]  # [T, DIN]
        rot = np.concatenate([xb[h * TOWN :], xb[: h * TOWN]], axis=0)
        m = dict(base)
        m["xT"] = np.ascontiguousarray(rot.T)  # [DIN, T]
        in_maps.append(m)
    return in_maps


def kernel(x, Wq, bq, Wk, bk, Wv, bv):
    from concourse import bass_utils

    x = np.ascontiguousarray(np.asarray(x, dtype=np.float32))
    nc = _get_nc()
    in_maps = _make_in_maps(x, Wq, bq, Wk, bk, Wv)

    res = bass_utils.run_bass_kernel_spmd(nc, in_maps, core_ids=list(range(NCORES)))

    bv = np.asarray(bv, np.float32).reshape(1, DV)
    outp = np.empty((B, T, DV), dtype=np.float32)
    for b in range(B):
        p0 = res.results[2 * b]["out"]  # natural query order (h=0)
        p1 = res.results[2 * b + 1]["out"]  # rotated by TOWN (h=1)
        p1 = np.concatenate([p1[TOWN:], p1[:TOWN]], axis=0)
        s = p0.astype(np.float64) + p1.astype(np.float64)
        outp[b] = (s[:, 0:DV] / s[:, DV : DV + 1] + bv).astype(np.float32)
    return outp


# revision 26
# speedup vs baseline: 1.0295x; 1.0295x over previous
"""Trainium2 Bass kernel for nn_AttentionLayer (4x2048x768, d_k=128, d_v=768).

Sharding (sequence-parallel over keys, data-parallel over batch):
8 cores; core c handles batch b=c//2 with KEY half h=c%2. Each core computes
q for ALL 2048 queries but k/v only for its own 1024 keys, then produces the
partial (unnormalized) attention numerator plus the partial softmax row sum:

    out_core[q, 0:768] = sum_{t in own half} exp(s_qt) * v[t, :]
    out_core[q, 768]   = sum_{t in own half} exp(s_qt)

The host adds the two partials of each batch and normalizes
(out = num/rowsum + bv) — an exact reassociation of the softmax.

x[b] is passed TRANSPOSED and t-rotated so the core's own key half is always
columns 0:1024 (one SPMD program serves all cores); query rows come back in
the rotated order and are un-rotated on the host.

Matmul dtype per stage: float32r (TF32-like, 1 PE cycle/row at N>=256;
rounding happens inside the PE on operand read — SBUF bytes stay fp32).
Set ATTN_MM_MODE=f32 for exact-fp32 matmuls (4 cycles/row).
"""

import sys

sys.path.insert(0, "/opt/trn_rl_repo")

import numpy as np

B, T, DIN, DK, DV = 4, 2048, 768, 128, 768
NCORES = 8
TOWN = 1024  # own keys per core
CH = DIN // 128  # 6 contraction chunks over d_in
TCH = TOWN // 128  # 8 own-key chunks
QCH = T // 128  # 16 query chunks (all queries)
SCALE = 1.0 / float(np.sqrt(DK))

import os as _os

_MODE = _os.environ.get("ATTN_MM_MODE", "f32r")  # "f32" | "f32r"
_R = _MODE == "f32r"

_CACHE = {}


def _build():
    from contextlib import ExitStack

    from concourse import bacc, mybir, tile

    f32 = mybir.dt.float32
    f32r = mybir.dt.float32r

    def rr(ap, on=True):
        return ap.bitcast(f32r) if (on and _R) else ap

    nc = bacc.Bacc("TRN2", target_bir_lowering=False, debug=False)

    xT = nc.dram_tensor("xT", [DIN, T], f32, kind="ExternalInput").ap()
    wq = nc.dram_tensor("wq", [DIN, DK], f32, kind="ExternalInput").ap()
    wk = nc.dram_tensor("wk", [DIN, DK], f32, kind="ExternalInput").ap()
    wv = nc.dram_tensor("wv", [DIN, DV], f32, kind="ExternalInput").ap()
    bq = nc.dram_tensor("bq", [DK, 1], f32, kind="ExternalInput").ap()
    bk = nc.dram_tensor("bk", [DK, 1], f32, kind="ExternalInput").ap()
    out = nc.dram_tensor("out", [T, DV + 1], f32, kind="ExternalOutput").ap()

    with tile.TileContext(nc) as tc, ExitStack() as ctx:
        consts = ctx.enter_context(tc.tile_pool(name="consts", bufs=1))
        persist = ctx.enter_context(tc.tile_pool(name="persist", bufs=1))
        wpool = ctx.enter_context(tc.tile_pool(name="wpool", bufs=1))
        xpool = ctx.enter_context(tc.tile_pool(name="xpool", bufs=1))
        out_pool = ctx.enter_context(tc.tile_pool(name="out_pool", bufs=4))
        ps_pool = ctx.enter_context(tc.tile_pool(name="ps", bufs=4, space="PSUM"))

        bq_sb = consts.tile([DK, 1], f32)
        bk_sb = consts.tile([DK, 1], f32)
        nc.gpsimd.dma_start(out=bq_sb[:], in_=bq)
        nc.gpsimd.dma_start(out=bk_sb[:], in_=bk)

        qT_sb = persist.tile([128, T], f32)  # [dk, q] all queries
        kT_sb = persist.tile([128, TOWN], f32)  # [dk, t-own]
        v_sb = persist.tile([128, TCH, DV + 2], f32)  # [t-part, chunk, dv|1|pad]
        pT_sb = persist.tile([128, TCH, T], f32)  # [t-part, chunk, q]

        nc.vector.memset(v_sb[:, :, DV : DV + 2], 1.0)

        xT_sb = xpool.tile([128, CH, T], f32)
        wq_sb = wpool.tile([128, CH, DK], f32)
        wk_sb = wpool.tile([128, CH, DK], f32)
        wv_sb = wpool.tile([128, CH, DV], f32)
        xT_r = xT.rearrange("(c p) t -> p c t", p=128)
        # DMA order: small weights on the scalar HWDGE queue; sync ring
        # carries (own-x chunks, wv, other-x chunks) in FIFO order so bytes
        # land in the order the PE consumes them.
        wq_r = wq.rearrange("(c p) k -> p c k", p=128)
        wk_r = wk.rearrange("(c p) k -> p c k", p=128)
        # per-chunk weight loads: MM #1 needs only wq[c0] (64KB), not all of wq
        for c in range(CH):
            nc.scalar.dma_start(out=rr(wq_sb[:, c]), in_=rr(wq_r[:, c]))
            nc.scalar.dma_start(out=rr(wk_sb[:, c]), in_=rr(wk_r[:, c]))
        for c in range(CH):
            for n0 in range(0, TOWN, 512):
                nc.sync.dma_start(
                    out=rr(xT_sb[:, c, n0 : n0 + 512]),
                    in_=rr(xT_r[:, c, n0 : n0 + 512]),
                )
        nc.sync.dma_start(
            out=rr(wv_sb[:]), in_=rr(wv.rearrange("(c p) k -> p c k", p=128))
        )
        for c in range(CH):
            nc.sync.dma_start(out=rr(xT_sb[:, c, TOWN:T]), in_=rr(xT_r[:, c, TOWN:T]))

        def emit_scores(qh):
            # scores^T per own-key chunk then P^T = exp(scale*s)
            for t in range(TCH):
                ps_s = ps_pool.tile([128, 1024], f32, tag="ps")
                for n0 in range(0, 1024, 512):
                    nc.tensor.matmul(
                        ps_s[:, n0 : n0 + 512],
                        rr(kT_sb[:, t * 128 : (t + 1) * 128]),
                        rr(qT_sb[:, qh * 1024 + n0 : qh * 1024 + n0 + 512]),
                        start=True,
                        stop=True,
                    )
                nc.scalar.activation(
                    rr(pT_sb[:, t, qh * 1024 : (qh + 1) * 1024]),
                    ps_s[:],
                    mybir.ActivationFunctionType.Exp,
                    scale=SCALE,
                )

        def emit_out(qh):
            # partial numerator + rowsum: out[qc] = sum_t P^T[t,qc].T @ [v|1];
            # copy+store each 512-col region as soon as its accumulation stops
            # so the final DMA overlaps the next region's matmuls
            for qc in range(qh * QCH // 2, (qh + 1) * QCH // 2):
                ps_o = ps_pool.tile([128, 1024], f32, tag="ps")
                o_sb = out_pool.tile([128, DV + 1], f32, tag="o")
                for n0, n1 in ((0, 512), (512, DV + 2)):
                    for t in range(TCH):
                        nc.tensor.matmul(
                            ps_o[:, n0:n1],
                            rr(pT_sb[:, t, qc * 128 : (qc + 1) * 128]),
                            rr(v_sb[:, t, n0:n1]),
                            start=(t == 0),
                            stop=(t == TCH - 1),
                        )
                    c1 = min(n1, DV + 1)
                    nc.vector.tensor_copy(o_sb[:, n0:c1], ps_o[:, n0:c1])
                    nc.sync.dma_start(
                        out=out[qc * 128 : (qc + 1) * 128, n0:c1],
                        in_=o_sb[:, n0:c1],
                    )

        # q own-half + k own, c-outer (PE consumes chunks as they stream)
        ps_q0 = ps_pool.tile([128, 1024], f32, tag="ps")
        ps_k = ps_pool.tile([128, 1024], f32, tag="ps")
        for c in range(CH):
            for n0 in range(0, TOWN, 512):
                nc.tensor.matmul(
                    ps_q0[:, n0 : n0 + 512],
                    rr(wq_sb[:, c, :]),
                    rr(xT_sb[:, c, n0 : n0 + 512]),
                    start=(c == 0),
                    stop=(c == CH - 1),
                )
                nc.tensor.matmul(
                    ps_k[:, n0 : n0 + 512],
                    rr(wk_sb[:, c, :]),
                    rr(xT_sb[:, c, n0 : n0 + 512]),
                    start=(c == 0),
                    stop=(c == CH - 1),
                )
        # split the bias-copies so scores t=0 unblocks as early as possible:
        # it needs only kT[:,0:128] and qT[:,0:512]
        for lo, hi in ((0, 128), (128, TOWN)):
            nc.scalar.activation(
                rr(kT_sb[:, lo:hi]),
                ps_k[:, lo:hi],
                mybir.ActivationFunctionType.Identity,
                bias=bk_sb[:],
            )
            nc.scalar.activation(
                rr(qT_sb[:, lo * 4 : min(hi * 4, TOWN)]),
                ps_q0[:, lo * 4 : min(hi * 4, TOWN)],
                mybir.ActivationFunctionType.Identity,
                bias=bq_sb[:],
            )

        # own-query scores need only q/k-own — run while wv/other-x stream
        emit_scores(0)

        # v for own keys
        for t in range(TCH):
            ps_v = ps_pool.tile([128, 1024], f32, tag="ps")
            for c in range(CH):
                for n0, n1 in ((0, 512), (512, DV)):
                    nc.tensor.matmul(
                        ps_v[:, n0:n1],
                        rr(xT_sb[:, c, t * 128 : (t + 1) * 128]),
                        rr(wv_sb[:, c, n0:n1]),
                        start=(c == 0),
                        stop=(c == CH - 1),
                    )
            nc.vector.tensor_copy(rr(v_sb[:, t, 0:DV]), ps_v[:, 0:DV])

        # q other-half (x tail has landed by now; its ACT overlaps out-qh0)
        ps_q1 = ps_pool.tile([128, 1024], f32, tag="ps")
        for c in range(CH):
            for n0 in range(0, TOWN, 512):
                nc.tensor.matmul(
                    ps_q1[:, n0 : n0 + 512],
                    rr(wq_sb[:, c, :]),
                    rr(xT_sb[:, c, TOWN + n0 : TOWN + n0 + 512]),
                    start=(c == 0),
                    stop=(c == CH - 1),
                )
        nc.scalar.activation(
            rr(qT_sb[:, TOWN:T]),
            ps_q1[:],
            mybir.ActivationFunctionType.Identity,
            bias=bq_sb[:],
        )

        # first output half while remaining bytes stream
        emit_out(0)

        emit_scores(1)
        emit_out(1)

    nc.compile()
    return nc


def _get_nc():
    if "nc" not in _CACHE:
        _CACHE["nc"] = _build()
    return _CACHE["nc"]


def _make_in_maps(x, Wq, bq, Wk, bk, Wv):
    base = {
        "wq": np.ascontiguousarray(Wq, dtype=np.float32),
        "wk": np.ascontiguousarray(Wk, dtype=np.float32),
        "wv": np.ascontiguousarray(Wv, dtype=np.float32),
        "bq": np.ascontiguousarray(np.asarray(bq, np.float32).reshape(DK, 1)),
        "bk": np.ascontiguousarray(np.asarray(bk, np.float32).reshape(DK, 1)),
    }
    in_maps = []
    for c in range(NCORES):
        b, h = c // 2, c % 2
        xb = x[b]  # [T, DIN]
        rot = np.concatenate([xb[h * TOWN :], xb[: h * TOWN]], axis=0)
        m = dict(base)
        m["xT"] = np.ascontiguousarray(rot.T)  # [DIN, T]
        in_maps.append(m)
    return in_maps


def kernel(x, Wq, bq, Wk, bk, Wv, bv):
    from concourse import bass_utils

    x = np.ascontiguousarray(np.asarray(x, dtype=np.float32))
    nc = _get_nc()
    in_maps = _make_in_maps(x, Wq, bq, Wk, bk, Wv)

    res = bass_utils.run_bass_kernel_spmd(nc, in_maps, core_ids=list(range(NCORES)))

    bv = np.asarray(bv, np.float32).reshape(1, DV)
    outp = np.empty((B, T, DV), dtype=np.float32)
    for b in range(B):
        p0 = res.results[2 * b]["out"]  # natural query order (h=0)
        p1 = res.results[2 * b + 1]["out"]  # rotated by TOWN (h=1)
        p1 = np.concatenate([p1[TOWN:], p1[:TOWN]], axis=0)
        s = p0.astype(np.float64) + p1.astype(np.float64)
        outp[b] = (s[:, 0:DV] / s[:, DV : DV + 1] + bv).astype(np.float32)
    return outp


# revision 27
# speedup vs baseline: 1.0621x; 1.0317x over previous
"""Trainium2 Bass kernel for nn_AttentionLayer (4x2048x768, d_k=128, d_v=768).

Sharding (sequence-parallel over keys, data-parallel over batch):
8 cores; core c handles batch b=c//2 with KEY half h=c%2. Each core computes
q for ALL 2048 queries but k/v only for its own 1024 keys, then produces the
partial (unnormalized) attention numerator plus the partial softmax row sum:

    out_core[q, 0:768] = sum_{t in own half} exp(s_qt) * v[t, :]
    out_core[q, 768]   = sum_{t in own half} exp(s_qt)

The host adds the two partials of each batch and normalizes
(out = num/rowsum + bv) — an exact reassociation of the softmax.

x[b] is passed TRANSPOSED and t-rotated so the core's own key half is always
columns 0:1024 (one SPMD program serves all cores); query rows come back in
the rotated order and are un-rotated on the host.

Matmul dtype per stage: float32r (TF32-like, 1 PE cycle/row at N>=256;
rounding happens inside the PE on operand read — SBUF bytes stay fp32).
Set ATTN_MM_MODE=f32 for exact-fp32 matmuls (4 cycles/row).
"""

import sys

sys.path.insert(0, "/opt/trn_rl_repo")

import numpy as np

B, T, DIN, DK, DV = 4, 2048, 768, 128, 768
NCORES = 8
TOWN = 1024  # own keys per core
CH = DIN // 128  # 6 contraction chunks over d_in
TCH = TOWN // 128  # 8 own-key chunks
QCH = T // 128  # 16 query chunks (all queries)
SCALE = 1.0 / float(np.sqrt(DK))

import os as _os

_MODE = _os.environ.get("ATTN_MM_MODE", "f32r")  # "f32" | "f32r"
_R = _MODE == "f32r"

_CACHE = {}


def _build():
    from contextlib import ExitStack

    from concourse import bacc, mybir, tile

    f32 = mybir.dt.float32
    f32r = mybir.dt.float32r

    def rr(ap, on=True):
        return ap.bitcast(f32r) if (on and _R) else ap

    nc = bacc.Bacc("TRN2", target_bir_lowering=False, debug=False)

    xT = nc.dram_tensor("xT", [DIN, T], f32, kind="ExternalInput").ap()
    wq = nc.dram_tensor("wq", [DIN, DK], f32, kind="ExternalInput").ap()
    wk = nc.dram_tensor("wk", [DIN, DK], f32, kind="ExternalInput").ap()
    wv = nc.dram_tensor("wv", [DIN, DV], f32, kind="ExternalInput").ap()
    bq = nc.dram_tensor("bq", [DK, 1], f32, kind="ExternalInput").ap()
    bk = nc.dram_tensor("bk", [DK, 1], f32, kind="ExternalInput").ap()
    out = nc.dram_tensor("out", [T, DV + 1], f32, kind="ExternalOutput").ap()

    with tile.TileContext(nc) as tc, ExitStack() as ctx:
        consts = ctx.enter_context(tc.tile_pool(name="consts", bufs=1))
        persist = ctx.enter_context(tc.tile_pool(name="persist", bufs=1))
        wpool = ctx.enter_context(tc.tile_pool(name="wpool", bufs=1))
        xpool = ctx.enter_context(tc.tile_pool(name="xpool", bufs=1))
        out_pool = ctx.enter_context(tc.tile_pool(name="out_pool", bufs=4))
        ps_pool = ctx.enter_context(tc.tile_pool(name="ps", bufs=4, space="PSUM"))

        bq_sb = consts.tile([DK, 1], f32)
        bk_sb = consts.tile([DK, 1], f32)
        nc.gpsimd.dma_start(out=bq_sb[:], in_=bq)
        nc.gpsimd.dma_start(out=bk_sb[:], in_=bk)

        qT_sb = persist.tile([128, T], f32)  # [dk, q] all queries
        kT_sb = persist.tile([128, TOWN], f32)  # [dk, t-own]
        v_sb = persist.tile([128, TCH, DV + 2], f32)  # [t-part, chunk, dv|1|pad]
        pT_sb = persist.tile([128, TCH, T], f32)  # [t-part, chunk, q]

        nc.vector.memset(v_sb[:, :, DV : DV + 2], 1.0)

        xT_sb = xpool.tile([128, CH, T], f32)
        wq_sb = wpool.tile([128, CH, DK], f32)
        wk_sb = wpool.tile([128, CH, DK], f32)
        wv_sb = wpool.tile([128, CH, DV], f32)
        xT_r = xT.rearrange("(c p) t -> p c t", p=128)
        # DMA order: small weights on the scalar HWDGE queue; sync ring
        # carries (own-x chunks, wv, other-x chunks) in FIFO order so bytes
        # land in the order the PE consumes them.
        nc.scalar.dma_start(
            out=rr(wq_sb[:]), in_=rr(wq.rearrange("(c p) k -> p c k", p=128))
        )
        nc.scalar.dma_start(
            out=rr(wk_sb[:]), in_=rr(wk.rearrange("(c p) k -> p c k", p=128))
        )
        for c in range(CH):
            for n0 in range(0, TOWN, 512):
                nc.sync.dma_start(
                    out=rr(xT_sb[:, c, n0 : n0 + 512]),
                    in_=rr(xT_r[:, c, n0 : n0 + 512]),
                )
        nc.sync.dma_start(
            out=rr(wv_sb[:]), in_=rr(wv.rearrange("(c p) k -> p c k", p=128))
        )
        for c in range(CH):
            nc.sync.dma_start(out=rr(xT_sb[:, c, TOWN:T]), in_=rr(xT_r[:, c, TOWN:T]))

        def emit_scores(qh):
            # scores^T per own-key chunk then P^T = exp(scale*s)
            for t in range(TCH):
                ps_s = ps_pool.tile([128, 1024], f32, tag="ps")
                for n0 in range(0, 1024, 512):
                    nc.tensor.matmul(
                        ps_s[:, n0 : n0 + 512],
                        rr(kT_sb[:, t * 128 : (t + 1) * 128]),
                        rr(qT_sb[:, qh * 1024 + n0 : qh * 1024 + n0 + 512]),
                        start=True,
                        stop=True,
                    )
                nc.scalar.activation(
                    rr(pT_sb[:, t, qh * 1024 : (qh + 1) * 1024]),
                    ps_s[:],
                    mybir.ActivationFunctionType.Exp,
                    scale=SCALE,
                )

        def emit_out(qh):
            # partial numerator + rowsum: out[qc] = sum_t P^T[t,qc].T @ [v|1];
            # copy+store each 512-col region as soon as its accumulation stops
            # so the final DMA overlaps the next region's matmuls
            for qc in range(qh * QCH // 2, (qh + 1) * QCH // 2):
                ps_o = ps_pool.tile([128, 1024], f32, tag="ps")
                o_sb = out_pool.tile([128, DV + 1], f32, tag="o")
                for n0, n1 in ((0, 512), (512, DV + 2)):
                    for t in range(TCH):
                        nc.tensor.matmul(
                            ps_o[:, n0:n1],
                            rr(pT_sb[:, t, qc * 128 : (qc + 1) * 128]),
                            rr(v_sb[:, t, n0:n1]),
                            start=(t == 0),
                            stop=(t == TCH - 1),
                        )
                    c1 = min(n1, DV + 1)
                    nc.vector.tensor_copy(o_sb[:, n0:c1], ps_o[:, n0:c1])
                    nc.sync.dma_start(
                        out=out[qc * 128 : (qc + 1) * 128, n0:c1],
                        in_=o_sb[:, n0:c1],
                    )

        # q own-half + k own, c-outer (PE consumes chunks as they stream)
        ps_q0 = ps_pool.tile([128, 1024], f32, tag="ps")
        ps_k = ps_pool.tile([128, 1024], f32, tag="ps")
        for c in range(CH):
            for n0 in range(0, TOWN, 512):
                nc.tensor.matmul(
                    ps_q0[:, n0 : n0 + 512],
                    rr(wq_sb[:, c, :]),
                    rr(xT_sb[:, c, n0 : n0 + 512]),
                    start=(c == 0),
                    stop=(c == CH - 1),
                )
                nc.tensor.matmul(
                    ps_k[:, n0 : n0 + 512],
                    rr(wk_sb[:, c, :]),
                    rr(xT_sb[:, c, n0 : n0 + 512]),
                    start=(c == 0),
                    stop=(c == CH - 1),
                )
        # split the bias-copies so scores t=0 unblocks as early as possible:
        # it needs only kT[:,0:128] and qT[:,0:512]
        for lo, hi in ((0, 128), (128, TOWN)):
            nc.scalar.activation(
                rr(kT_sb[:, lo:hi]),
                ps_k[:, lo:hi],
                mybir.ActivationFunctionType.Identity,
                bias=bk_sb[:],
            )
            nc.scalar.activation(
                rr(qT_sb[:, lo * 4 : min(hi * 4, TOWN)]),
                ps_q0[:, lo * 4 : min(hi * 4, TOWN)],
                mybir.ActivationFunctionType.Identity,
                bias=bq_sb[:],
            )

        # own-query scores need only q/k-own — run while wv/other-x stream
        emit_scores(0)

        # v for own keys
        for t in range(TCH):
            ps_v = ps_pool.tile([128, 1024], f32, tag="ps")
            for c in range(CH):
                for n0, n1 in ((0, 512), (512, DV)):
                    nc.tensor.matmul(
                        ps_v[:, n0:n1],
                        rr(xT_sb[:, c, t * 128 : (t + 1) * 128]),
                        rr(wv_sb[:, c, n0:n1]),
                        start=(c == 0),
                        stop=(c == CH - 1),
                    )
            nc.vector.tensor_copy(rr(v_sb[:, t, 0:DV]), ps_v[:, 0:DV])

        # q other-half (x tail has landed by now; its ACT overlaps out-qh0)
        ps_q1 = ps_pool.tile([128, 1024], f32, tag="ps")
        for c in range(CH):
            for n0 in range(0, TOWN, 512):
                nc.tensor.matmul(
                    ps_q1[:, n0 : n0 + 512],
                    rr(wq_sb[:, c, :]),
                    rr(xT_sb[:, c, TOWN + n0 : TOWN + n0 + 512]),
                    start=(c == 0),
                    stop=(c == CH - 1),
                )
        nc.scalar.activation(
            rr(qT_sb[:, TOWN:T]),
            ps_q1[:],
            mybir.ActivationFunctionType.Identity,
            bias=bq_sb[:],
        )

        # first output half while remaining bytes stream
        emit_out(0)

        emit_scores(1)
        emit_out(1)

    nc.compile()
    return nc


def _get_nc():
    if "nc" not in _CACHE:
        _CACHE["nc"] = _build()
    return _CACHE["nc"]


def _make_in_maps(x, Wq, bq, Wk, bk, Wv):
    base = {
        "wq": np.ascontiguousarray(Wq, dtype=np.float32),
        "wk": np.ascontiguousarray(Wk, dtype=np.float32),
        "wv": np.ascontiguousarray(Wv, dtype=np.float32),
        "bq": np.ascontiguousarray(np.asarray(bq, np.float32).reshape(DK, 1)),
        "bk": np.ascontiguousarray(np.asarray(bk, np.float32).reshape(DK, 1)),
    }
    in_maps = []
    for c in range(NCORES):
        b, h = c // 2, c % 2
        xb = x[b]  # [T, DIN]
        rot = np.concatenate([xb[h * TOWN :], xb[: h * TOWN]], axis=0)
        m = dict(base)
        m["xT"] = np.ascontiguousarray(rot.T)  # [DIN, T]
        in_maps.append(m)
    return in_maps


def kernel(x, Wq, bq, Wk, bk, Wv, bv):
    from concourse import bass_utils

    x = np.ascontiguousarray(np.asarray(x, dtype=np.float32))
    nc = _get_nc()
    in_maps = _make_in_maps(x, Wq, bq, Wk, bk, Wv)

    res = bass_utils.run_bass_kernel_spmd(nc, in_maps, core_ids=list(range(NCORES)))

    bv = np.asarray(bv, np.float32).reshape(1, DV)
    outp = np.empty((B, T, DV), dtype=np.float32)
    for b in range(B):
        p0 = res.results[2 * b]["out"]  # natural query order (h=0)
        p1 = res.results[2 * b + 1]["out"]  # rotated by TOWN (h=1)
        p1 = np.concatenate([p1[TOWN:], p1[:TOWN]], axis=0)
        s = p0.astype(np.float64) + p1.astype(np.float64)
        outp[b] = (s[:, 0:DV] / s[:, DV : DV + 1] + bv).astype(np.float32)
    return outp
